# revision 2
# baseline (speedup 1.0000x reference)
"""Self-contained GAT (3x GATConv + pool + dense tail) on 8 trn2 NeuronCores."""

import os
import numpy as np

import concourse.bass as bass
import concourse.bacc as bacc
import concourse.mybir as mybir
import concourse.tile as tile
from concourse._compat import cdiv
from concourse.masks import make_identity

P = 128
NCORES = 8
NEG_SLOPE = 0.2
NEG_BIG = -1.0e30
CALL_CHUNKS = 5
MAXSLOT = 12

F32 = mybir.dt.float32
I32 = mybir.dt.int32
I16 = mybir.dt.int16

TAIL_SHAPES = dict(d1=(512, 256), d2=(256, 128), d3=(128, 64), mean=(256, 64),
                   d4=(64, 64), d5=(64, 128), d6=(128, 256), d7=(256, 128))


def row_floats(d):
    return int(cdiv(d + 2, 64) * 64)


def _wrap_idx(flat):
    n = len(flat)
    assert n % 16 == 0
    w = np.zeros((P, n // 16), np.int16)
    k = np.arange(n)
    w[(k % 16)[None, :] + 16 * np.arange(8)[:, None], (k // 16)[None, :]] = \
        np.asarray(flat, np.int16)[None, :]
    return w


def preprocess(x, edge_index, batch, G, layer_dims):
    N, F_IN = x.shape
    src = np.asarray(edge_index[0]).astype(np.int64)
    dst = np.asarray(edge_index[1]).astype(np.int64)
    batch = np.asarray(batch).astype(np.int64)

    cnt = np.bincount(batch, minlength=G).astype(np.int64)
    blk = np.maximum(1, np.ceil(cnt / P).astype(np.int64)) * P
    gstart_pad = np.concatenate([[0], np.cumsum(blk)])
    T = int(cdiv(gstart_pad[-1] // P, NCORES * 2) * NCORES * 2)  # mult of 16
    NT = T * P
    TOWN = T // NCORES
    gstart_real = np.concatenate([[0], np.cumsum(cnt)])
    node_map = np.zeros(N, np.int64)
    for g in range(G):
        a, b = gstart_real[g], gstart_real[g + 1]
        node_map[a:b] = gstart_pad[g] + np.arange(b - a)

    T_lo = T // 2
    HALF = T_lo * P
    padA_row = HALF                    # phys rows: [lo nodes | padA | hi nodes | padB]
    padB_row = NT + 1
    NROWS = NT + 2
    assert HALF + 1 <= 32768 and (NT - HALF) + 1 <= 32768, (HALF, NT)

    sl = np.arange(N)
    srcp = node_map[np.concatenate([src, sl])]
    dstp = node_map[np.concatenate([dst, sl])]
    dst_tile = dstp // P
    order = np.lexsort((dstp, dst_tile))
    srcp, dstp, dst_tile = srcp[order], dstp[order], dst_tile[order]
    lo_mask = srcp < HALF
    tstart = np.searchsorted(dst_tile, np.arange(T + 1))

    K_lo = np.ones(TOWN, np.int64)
    K_hi = np.ones(TOWN, np.int64)
    per_tile = []
    for t in range(T):
        a, b = tstart[t], tstart[t + 1]
        m = lo_mask[a:b]
        per_tile.append((srcp[a:b][m], srcp[a:b][~m], dstp[a:b][m], dstp[a:b][~m]))
        lt = t % TOWN
        K_lo[lt] = max(K_lo[lt], cdiv(len(per_tile[-1][0]), P))
        K_hi[lt] = max(K_hi[lt], cdiv(len(per_tile[-1][1]), P))

    nchunks = int((K_lo + K_hi).sum())
    idx16 = np.zeros((NCORES, P, nchunks * 8), np.int16)
    dstloc = np.zeros((NCORES, P, nchunks), np.float32)
    for c in range(NCORES):
        ch = 0
        for lt in range(TOWN):
            t = c * TOWN + lt
            s_lo, s_hi, d_lo, d_hi = per_tile[t]
            for (s_arr, d_arr, K, is_lo) in ((s_lo, d_lo, int(K_lo[lt]), True),
                                             (s_hi, d_hi, int(K_hi[lt]), False)):
                ns = K * P
                if is_lo:
                    vidx = np.full(ns, padA_row, np.int64)
                    vidx[: len(s_arr)] = s_arr
                else:
                    vidx = np.full(ns, padB_row - (HALF + 1), np.int64)
                    vidx[: len(s_arr)] = s_arr + 1 - (HALF + 1)
                dl = np.zeros(ns, np.int64)
                dl[: len(d_arr)] = d_arr - t * P
                idx16[c, :, ch * 8:(ch + K) * 8] = _wrap_idx(vidx)
                dstloc[c, :, ch:ch + K] = dl.reshape(K, P).T.astype(np.float32)
                ch += K
        assert ch == nchunks

    xT = np.zeros((F_IN, NT), np.float32)
    xT[:, node_map] = np.asarray(x, np.float32).T

    tile_graph = np.searchsorted(gstart_pad, np.arange(T) * P, side="right") - 1
    tile_graph = np.minimum(tile_graph, G - 1)
    real = np.zeros(NT, np.float32)
    real[node_map] = 1.0
    padmask = real.reshape(NCORES, TOWN, P)

    # per-core graph slots: slot s of core c covers graph slot_graph[c,s];
    # tiles of that graph on that core = slot_lt[c][s] (contiguous list)
    slot_graph = np.full((NCORES, MAXSLOT), -1, np.int64)
    tile_slot = np.zeros((NCORES, TOWN), np.int64)
    for c in range(NCORES):
        gs = []
        for lt in range(TOWN):
            g = int(tile_graph[c * TOWN + lt])
            if g not in gs:
                gs.append(g)
            tile_slot[c, lt] = gs.index(g)
        assert len(gs) <= MAXSLOT, len(gs)
        slot_graph[c, :len(gs)] = gs

    # slot-tile masks as data: [MAXSLOT, TOWN] {1,0} and offs for max
    slot_mask = np.zeros((NCORES, MAXSLOT, TOWN), np.float32)
    for c in range(NCORES):
        for lt in range(TOWN):
            slot_mask[c, tile_slot[c, lt], lt] = 1.0

    # graph -> up to two (core, slot) sources (build-time constants, global)
    gsrc = []
    for g in range(G):
        locs = [(c, s) for c in range(NCORES) for s in range(MAXSLOT)
                if slot_graph[c, s] == g]
        assert 1 <= len(locs) <= 2, (g, locs)
        if len(locs) == 1:
            locs = locs * 2
        gsrc.append(locs)

    recip_cnt = (1.0 / np.maximum(cnt, 1.0)).astype(np.float32)

    return dict(N=N, F_IN=F_IN, G=G, NT=NT, T=T, TOWN=TOWN, HALF=HALF,
                NROWS=NROWS, padA=padA_row, padB=padB_row, layer_dims=layer_dims,
                K_lo=K_lo, K_hi=K_hi, nchunks=nchunks, idx16=idx16,
                dstloc=dstloc, xT=xT, node_map=node_map, padmask=padmask,
                slot_mask=slot_mask, gsrc=gsrc, recip_cnt=recip_cnt)


def make_inputs(pp, weights):
    layer_dims = pp["layer_dims"]
    ins_shared = {"xT": pp["xT"]}
    for li, (di, do) in enumerate(layer_dims):
        W = np.asarray(weights[f"W{li+1}"], np.float32)
        a_s = np.asarray(weights[f"asrc{li+1}"], np.float32)
        a_d = np.asarray(weights[f"adst{li+1}"], np.float32)
        b = np.asarray(weights[f"b{li+1}"], np.float32)
        wext = np.concatenate([W, (W @ a_s)[:, None], (W @ a_d)[:, None]], axis=1)
        ins_shared[f"wext{li}"] = wext.astype(np.float32)
        ins_shared[f"bias{li}"] = b[None, :].astype(np.float32)
        pr = np.zeros((2, row_floats(do)), np.float32)
        pr[:, do + 1] = NEG_BIG
        ins_shared[f"padrows{li}"] = pr
    for name, (a, bsz) in TAIL_SHAPES.items():
        ins_shared[name + "_w"] = np.asarray(weights[name + "_w"], np.float32)
        ins_shared[name + "_b"] = np.asarray(weights[name + "_b"], np.float32)[:, None]
    ins_shared["recip_cnt"] = pp["recip_cnt"][None, :]
    onesrow = np.ones((1, P), np.float32)
    ins_shared["ones_row"] = onesrow

    in_maps = []
    for c in range(NCORES):
        m = dict(ins_shared)
        m["idx16"] = pp["idx16"][c]
        m["dstloc"] = pp["dstloc"][c]
        m["padmask"] = pp["padmask"][c]
        m["padoff"] = (-3.0e38 * (1.0 - pp["padmask"][c])).astype(np.float32)
        m["slot_mask"] = pp["slot_mask"][c]
        m["slot_off"] = (-3.0e38 * (1.0 - pp["slot_mask"][c])).astype(np.float32)
        in_maps.append(m)
    return in_maps


def build_kernel(pp):
    layer_dims = pp["layer_dims"]
    NT, T, TOWN = pp["NT"], pp["T"], pp["TOWN"]
    HALF, NROWS = pp["HALF"], pp["NROWS"]
    K_lo, K_hi, nchunks = pp["K_lo"], pp["K_hi"], pp["nchunks"]
    G = pp["G"]
    F_IN = pp["F_IN"]
    gsrc = pp["gsrc"]
    d3out = layer_dims[-1][1]          # 256
    NH3 = d3out // P                   # halves in layer-3 output (2)

    nc = bacc.Bacc("TRN2", num_devices=NCORES, num_swdge_queues=4)
    dp = nc.declare_dram_parameter

    t_xT = dp("xT", [F_IN, NT], F32, isOutput=False)
    t_idx = dp("idx16", [P, nchunks * 8], I16, isOutput=False)
    t_dstloc = dp("dstloc", [P, nchunks], F32, isOutput=False)
    t_ones = dp("ones_row", [1, P], F32, isOutput=False)
    t_wext, t_bias, t_padrows = [], [], []
    for li, (di, do) in enumerate(layer_dims):
        t_wext.append(dp(f"wext{li}", [di, do + 2], F32, isOutput=False))
        t_bias.append(dp(f"bias{li}", [1, do], F32, isOutput=False))
        t_padrows.append(dp(f"padrows{li}", [2, row_floats(do)], F32, isOutput=False))
    t_padmask = dp("padmask", [TOWN, P], F32, isOutput=False)
    t_padoff = dp("padoff", [TOWN, P], F32, isOutput=False)
    t_slotm = dp("slot_mask", [MAXSLOT, TOWN], F32, isOutput=False)
    t_sloto = dp("slot_off", [MAXSLOT, TOWN], F32, isOutput=False)
    t_rcnt = dp("recip_cnt", [1, G], F32, isOutput=False)
    t_tail = {}
    for name, (a, b) in TAIL_SHAPES.items():
        t_tail[name + "_w"] = dp(name + "_w", [a, b], F32, isOutput=False)
        t_tail[name + "_b"] = dp(name + "_b", [b, 1], F32, isOutput=False)
    t_out = dp("z_out", [G, 128], F32, isOutput=True)

    NRA = HALF + 1
    NRB = NT - HALF + 1
    tablesA = [nc.dram_tensor(f"tableA{li}", [NRA, row_floats(do)], F32)
               for li, (di, do) in enumerate(layer_dims)]
    tablesB = [nc.dram_tensor(f"tableB{li}", [NRB, row_floats(do)], F32)
               for li, (di, do) in enumerate(layer_dims)]
    s2tab = [nc.dram_tensor(f"s2tab{li}", [T, P], F32) for li in range(len(layer_dims))]
    s2own = [nc.dram_tensor(f"s2own{li}", [TOWN, P], F32) for li in range(len(layer_dims))]
    xT_shard = [nc.dram_tensor(f"xT_shard{li}", [do, TOWN * P], F32)
                for li, (di, do) in enumerate(layer_dims[:-1])]
    xT_blk = [nc.dram_tensor(f"xT_blk{li}", [NCORES * do, TOWN * P], F32,
                             addr_space="Shared")
              for li, (di, do) in enumerate(layer_dims[:-1])]
    POOLW = 2 * NH3 * MAXSLOT          # kind(max,sum) x half x slot
    pool_shard = nc.dram_tensor("pool_shard", [P, POOLW], F32)
    pool_all = nc.dram_tensor("pool_all", [NCORES * P, POOLW], F32, addr_space="Shared")

    RG = [list(range(NCORES))]

    with tile.TileContext(nc) as tc:
        with (
            tc.tile_pool(name="const", bufs=1) as cb,
            tc.tile_pool(name="sbuf", bufs=3) as sb,
            tc.tile_pool(name="gat", bufs=4) as sg,
            tc.tile_pool(name="msk", bufs=4) as sm,
            tc.tile_pool(name="psA", bufs=2, space="PSUM") as psA,
            tc.tile_pool(name="psN", bufs=2, space="PSUM") as psN,
            tc.tile_pool(name="psS", bufs=3, space="PSUM") as psS,
        ):
            # ------------- constants -------------
            iota_i = cb.tile([P, P], I32)
            nc.gpsimd.iota(iota_i[:], pattern=[[1, P]], base=0, channel_multiplier=0)
            iota_f = cb.tile([P, P], F32)
            nc.vector.tensor_copy(out=iota_f[:], in_=iota_i[:])
            ident = cb.tile([P, P], F32)
            make_identity(nc, ident[:])
            ones_row = cb.tile([1, P], F32)
            nc.sync.dma_start(out=ones_row[:], in_=t_ones[:])

            idx_sb = cb.tile([P, nchunks * 8], I16)
            nc.sync.dma_start(out=idx_sb[:], in_=t_idx[:])
            dstloc_sb = cb.tile([P, nchunks], F32)
            nc.sync.dma_start(out=dstloc_sb[:], in_=t_dstloc[:])

            wext_sb, bias_bc = [], []
            for li, (di, do) in enumerate(layer_dims):
                w = cb.tile([di, do + 2], F32, tag=f"wext{li}")
                nc.sync.dma_start(out=w[:], in_=t_wext[li][:])
                wext_sb.append(w)
                # bias broadcast [P, do] via K=1 matmul
                brow = cb.tile([1, do], F32, tag=f"brow{li}")
                nc.sync.dma_start(out=brow[:], in_=t_bias[li][:])
                bps = psS.tile([P, do], F32, space="PSUM", tag="ps")
                nc.tensor.matmul(out=bps[:], lhsT=ones_row[:], rhs=brow[:],
                                 start=True, stop=True)
                bb = cb.tile([P, do], F32, tag=f"biasbc{li}")
                nc.vector.tensor_copy(out=bb[:], in_=bps[:])
                bias_bc.append(bb)

            # pooling stages
            rmax_stage = cb.tile([P, NH3, TOWN], F32)
            rsum_stage = cb.tile([P, NH3, TOWN], F32)

            # ================= layers =================
            NLAYERS = int(os.environ.get("GAT_LAYERS", len(layer_dims)))
            NO_B = bool(os.environ.get("GAT_NO_B"))
            for li, (di, do) in enumerate(layer_dims[:NLAYERS]):
                ROW = row_floats(do)
                tabA, tabB = tablesA[li], tablesB[li]
                last = (li == len(layer_dims) - 1) and NLAYERS == len(layer_dims)

                # ---------- phase A ----------
                padrow_sb = sb.tile([2, ROW], F32, tag="padrow")
                nc.sync.dma_start(out=padrow_sb[:], in_=t_padrows[li][:])
                nc.sync.dma_start(out=tabA[HALF:HALF + 1, :], in_=padrow_sb[0:1, :])
                nc.sync.dma_start(out=tabB[NT - HALF:NT - HALF + 1, :], in_=padrow_sb[1:2, :])

                s2big = cb.tile([P, T], F32, tag=f"s2big")
                for t in range(T):
                    lhsT = sb.tile([di, P], F32, tag="lhsT")
                    if li == 0:
                        nc.sync.dma_start(out=lhsT[:], in_=t_xT[:, t * P:(t + 1) * P])
                    else:
                        cblk = t // TOWN
                        nc.sync.dma_start(
                            out=lhsT[:],
                            in_=xT_blk[li - 1][cblk * di:(cblk + 1) * di,
                                               (t % TOWN) * P:(t % TOWN + 1) * P])
                    hp = psA.tile([P, do + 2], F32, space="PSUM", tag="hpsum")
                    nc.tensor.matmul(out=hp[:], lhsT=lhsT[:], rhs=wext_sb[li][:],
                                     start=True, stop=True)
                    ttile = sb.tile([P, do + 2], F32, tag="ttile")
                    nc.vector.tensor_copy(out=ttile[:, 0:do], in_=hp[:, 0:do])
                    nc.gpsimd.memset(ttile[:, do:do + 1], 1.0)
                    nc.vector.tensor_copy(out=ttile[:, do + 1:do + 2],
                                          in_=hp[:, do:do + 1])
                    nc.vector.tensor_copy(out=s2big[:, t:t + 1], in_=hp[:, do + 1:do + 2])
                    if t * P < HALF:
                        nc.sync.dma_start(out=tabA[t * P:t * P + P, 0:do + 2], in_=ttile[:])
                    else:
                        nc.sync.dma_start(out=tabB[t * P - HALF:t * P - HALF + P, 0:do + 2],
                                          in_=ttile[:])

                # s2: transpose batches of 128 tiles -> s2tab [T, P]
                for bt in range(cdiv(T, P)):
                    w = min(P, T - bt * P)
                    tp = psS.tile([P, P], F32, space="PSUM", tag="ps")
                    nc.tensor.transpose(out=tp[:w, :], in_=s2big[:, bt * P:bt * P + w],
                                        identity=ident[:])
                    ts_sb = sb.tile([P, P], F32, tag="s2Tsb")
                    nc.vector.tensor_copy(out=ts_sb[:w, :], in_=tp[:w, :])
                    nc.sync.dma_start(out=s2tab[li][bt * P:bt * P + w, :],
                                      in_=ts_sb[:w, :])
                if os.environ.get("GAT_NO_RS"):
                    tmp_rs = sb.tile([TOWN, P], F32, tag="tmprs")
                    nc.sync.dma_start(out=tmp_rs[:], in_=s2tab[li][0:TOWN, :])
                    nc.sync.dma_start(out=s2own[li][:, :], in_=tmp_rs[:])
                else:
                    nc.gpsimd.collective_compute(
                        "ReduceScatter", mybir.AluOpType.max, replica_groups=RG,
                        ins=[s2tab[li][:, :]], outs=[s2own[li][:, :]])
                s2flat = cb.tile([1, TOWN * P], F32, tag=f"s2flat")
                nc.sync.dma_start(out=s2flat[:],
                                  in_=s2own[li][:, :].rearrange("a b -> (a b)").unsqueeze(0))

                # ---------- phase B ----------
                if NO_B:
                    continue
                ch = 0
                for lt in range(TOWN):
                    # S2B [P, P]: s2 of tile's nodes along free dim, bcast partitions
                    sps = psS.tile([P, P], F32, space="PSUM", tag="ps")
                    nc.tensor.matmul(out=sps[:], lhsT=ones_row[:],
                                     rhs=s2flat[:, lt * P:(lt + 1) * P],
                                     start=True, stop=True)
                    S2B = sm.tile([P, P], F32, tag="S2Bsb")
                    nc.vector.tensor_copy(out=S2B[:], in_=sps[:])

                    num = psN.tile([P, do + 1], F32, space="PSUM", tag="num")
                    ktot = int(K_lo[lt] + K_hi[lt])
                    done = 0
                    for half in (0, 1):
                        K = int(K_lo[lt]) if half == 0 else int(K_hi[lt])
                        view = tabA[:, :] if half == 0 else tabB[:, :]
                        for c0 in range(0, K, CALL_CHUNKS):
                            k = min(CALL_CHUNKS, K - c0)
                            G_t = sg.tile([P, k, ROW], F32, tag="G")
                            nc.gpsimd.dma_gather(
                                G_t[:], view, idx_sb[:, ch * 8:(ch + k) * 8],
                                k * P, k * P, ROW,
                                queue_num=(ch // 2) % 4)
                            s2e = sm.tile([P, k], F32, tag="s2e")
                            masks = []
                            for j in range(k):
                                mk = sm.tile([P, P], F32, tag=f"mask{j}")
                                nc.vector.tensor_scalar(
                                    out=mk[:], in0=iota_f[:],
                                    scalar1=dstloc_sb[:, ch + j:ch + j + 1],
                                    scalar2=None, op0=mybir.AluOpType.is_equal)
                                scr = sm.tile([P, P], F32, tag="scr")
                                nc.vector.scalar_tensor_tensor(
                                    out=scr[:], in0=mk[:], scalar=1.0, in1=S2B[:],
                                    op0=mybir.AluOpType.mult,
                                    op1=mybir.AluOpType.mult,
                                    accum_out=s2e[:, j:j + 1])
                                masks.append(mk)
                            raw = sm.tile([P, k], F32, tag="raw")
                            nc.vector.tensor_add(
                                out=raw[:], in0=G_t[:, :, do + 1], in1=s2e[:])
                            r2 = sm.tile([P, k], F32, tag="r2")
                            nc.vector.tensor_scalar_mul(r2[:], raw[:], NEG_SLOPE)
                            lr = sm.tile([P, k], F32, tag="lr")
                            nc.vector.tensor_max(out=lr[:], in0=raw[:], in1=r2[:])
                            ex = sm.tile([P, k], F32, tag="ex")
                            nc.scalar.activation(ex[:], lr[:],
                                                 mybir.ActivationFunctionType.Exp)
                            for j in range(k):
                                sel = sm.tile([P, P], F32, tag=f"sel{j}")
                                if (ch + j) % 2 == 0:
                                    nc.scalar.activation(
                                        sel[:], masks[j][:],
                                        mybir.ActivationFunctionType.Copy,
                                        scale=ex[:, j:j + 1])
                                else:
                                    nc.vector.tensor_scalar(
                                        out=sel[:], in0=masks[j][:],
                                        scalar1=ex[:, j:j + 1], scalar2=None,
                                        op0=mybir.AluOpType.mult)
                                nc.tensor.matmul(out=num[:], lhsT=sel[:],
                                                 rhs=G_t[:, j, 0:do + 1],
                                                 start=(done == 0),
                                                 stop=(done == ktot - 1))
                                done += 1
                                ch += 1
                    # epilogue
                    den = sm.tile([P, 1], F32, tag="den")
                    nc.vector.tensor_scalar(out=den[:], in0=num[:, do:do + 1],
                                            scalar1=1.0e-30, scalar2=None,
                                            op0=mybir.AluOpType.max)
                    rden = sm.tile([P, 1], F32, tag="rden")
                    nc.vector.reciprocal(out=rden[:], in_=den[:])
                    outb = sb.tile([P, do], F32, tag="outb")
                    nc.vector.tensor_scalar(out=outb[:], in0=num[:, 0:do],
                                            scalar1=rden[:], scalar2=None,
                                            op0=mybir.AluOpType.mult)
                    outb2 = sb.tile([P, do], F32, tag="outb2")
                    nc.vector.tensor_add(out=outb2[:], in0=outb[:], in1=bias_bc[li][:])
                    for hf in range(cdiv(do, P)):
                        w = min(P, do - hf * P)
                        tp = psS.tile([P, P], F32, space="PSUM", tag="ps")
                        nc.tensor.transpose(out=tp[:w, :],
                                            in_=outb2[:, hf * P:hf * P + w],
                                            identity=ident[:])
                        oT = sb.tile([P, P], F32, tag="oTsb")
                        nc.vector.tensor_copy(out=oT[:w, :], in_=tp[:w, :])
                        if not last:
                            nc.sync.dma_start(
                                out=xT_shard[li][hf * P:hf * P + w,
                                                 lt * P:(lt + 1) * P],
                                in_=oT[:w, :])
                        else:
                            # pooling: mask pads, per-tile max/sum over nodes
                            pmrow = sb.tile([1, P], F32, tag="pmrow")
                            nc.sync.dma_start(out=pmrow[:], in_=t_padmask[lt:lt + 1, :])
                            porow = sb.tile([1, P], F32, tag="porow")
                            nc.sync.dma_start(out=porow[:], in_=t_padoff[lt:lt + 1, :])
                            pmb = psS.tile([P, P], F32, space="PSUM", tag="ps")
                            nc.tensor.matmul(out=pmb[:], lhsT=ones_row[:],
                                             rhs=pmrow[:], start=True, stop=True)
                            pob = psS.tile([P, P], F32, space="PSUM", tag="ps")
                            nc.tensor.matmul(out=pob[:], lhsT=ones_row[:],
                                             rhs=porow[:], start=True, stop=True)
                            hz = sb.tile([P, P], F32, tag="hz")
                            nc.vector.tensor_mul(out=hz[:], in0=oT[:], in1=pmb[:])
                            nc.vector.tensor_reduce(
                                out=rsum_stage[:, hf, lt:lt + 1], in_=hz[:],
                                axis=mybir.AxisListType.X, op=mybir.AluOpType.add)
                            hm = sb.tile([P, P], F32, tag="hm")
                            nc.vector.tensor_add(out=hm[:], in0=hz[:], in1=pob[:])
                            nc.vector.tensor_reduce(
                                out=rmax_stage[:, hf, lt:lt + 1], in_=hm[:],
                                axis=mybir.AxisListType.X, op=mybir.AluOpType.max)

                if not last:
                    if os.environ.get("GAT_NO_AG"):
                        tmp_ag = sb.tile([P, TOWN * P], F32, tag="tmpag")
                        for cc in range(NCORES):
                            nc.sync.dma_start(out=tmp_ag[:do, :], in_=xT_shard[li][:, :])
                            nc.sync.dma_start(
                                out=xT_blk[li][cc * do:(cc + 1) * do, :],
                                in_=tmp_ag[:do, :])
                    else:
                        nc.gpsimd.collective_compute(
                            "AllGather", mybir.AluOpType.bypass, replica_groups=RG,
                            ins=[xT_shard[li][:, :]], outs=[xT_blk[li][:, :]])

            # ================= pooling combine + tail =================
            if NLAYERS < len(layer_dims) or NO_B:
                zstub = sb.tile([G, 128], F32, tag="zstub")
                nc.vector.memset(zstub[:], 0.0)
                nc.sync.dma_start(out=t_out[:, :], in_=zstub[:])
            else_skip = False
            if not (NLAYERS < len(layer_dims) or NO_B):
                # per-slot combine over own tiles
                pool_loc = cb.tile([P, 2, NH3, MAXSLOT], F32)   # kind, half, slot
                for s in range(MAXSLOT):
                    smrow = sb.tile([1, TOWN], F32, tag="smrow")
                    nc.sync.dma_start(out=smrow[:], in_=t_slotm[s:s + 1, :])
                    sorow = sb.tile([1, TOWN], F32, tag="sorow")
                    nc.sync.dma_start(out=sorow[:], in_=t_sloto[s:s + 1, :])
                    smb = psS.tile([P, TOWN], F32, space="PSUM", tag="ps")
                    nc.tensor.matmul(out=smb[:], lhsT=ones_row[:], rhs=smrow[:],
                                     start=True, stop=True)
                    sob = psS.tile([P, TOWN], F32, space="PSUM", tag="ps")
                    nc.tensor.matmul(out=sob[:], lhsT=ones_row[:], rhs=sorow[:],
                                     start=True, stop=True)
                    for hf in range(NH3):
                        mz = sb.tile([P, TOWN], F32, tag="mz")
                        nc.vector.tensor_mul(out=mz[:], in0=rsum_stage[:, hf, :], in1=smb[:])
                        nc.vector.tensor_reduce(
                            out=pool_loc[:, 1, hf, s:s + 1], in_=mz[:],
                            axis=mybir.AxisListType.X, op=mybir.AluOpType.add)
                        mm = sb.tile([P, TOWN], F32, tag="mm")
                        nc.vector.tensor_mul(out=mm[:], in0=rmax_stage[:, hf, :], in1=smb[:])
                        mm2 = sb.tile([P, TOWN], F32, tag="mm2")
                        nc.vector.tensor_add(out=mm2[:], in0=mm[:], in1=sob[:])
                        nc.vector.tensor_reduce(
                            out=pool_loc[:, 0, hf, s:s + 1], in_=mm2[:],
                            axis=mybir.AxisListType.X, op=mybir.AluOpType.max)
                pl_sb = sb.tile([P, 2 * NH3 * MAXSLOT], F32, tag="plf")
                nc.vector.tensor_copy(out=pl_sb[:], in_=pool_loc[:].rearrange("p a b c -> p (a b c)"))
                nc.sync.dma_start(out=pool_shard[:, :], in_=pl_sb[:])
                if os.environ.get("GAT_NO_POOLAG"):
                    for cc in range(NCORES):
                        tmp_pg = sb.tile([P, POOLW], F32, tag="tmppg")
                        nc.sync.dma_start(out=tmp_pg[:], in_=pool_shard[:, :])
                        nc.sync.dma_start(out=pool_all[cc * P:(cc + 1) * P, :], in_=tmp_pg[:])
                else:
                    nc.gpsimd.collective_compute(
                        "AllGather", mybir.AluOpType.bypass, replica_groups=RG,
                        ins=[pool_shard[:, :]], outs=[pool_all[:, :]])
                POOLW = 2 * NH3 * MAXSLOT
                pa = cb.tile([P, NCORES, POOLW], F32)
                nc.sync.dma_start(
                    out=pa[:], in_=pool_all[:, :].rearrange("(c p) w -> p c w", p=P))

                # per-graph combine (build-time (core,slot) pairs)
                x1T = cb.tile([P, NH3, G], F32)
                x2T = cb.tile([P, NH3, G], F32)
                for g in range(G):
                    (cA, sA), (cB, sB) = gsrc[g]
                    for hf in range(NH3):
                        iA = 0 * NH3 * MAXSLOT + hf * MAXSLOT + sA
                        iB = 0 * NH3 * MAXSLOT + hf * MAXSLOT + sB
                        nc.vector.tensor_max(out=x1T[:, hf, g:g + 1],
                                             in0=pa[:, cA, iA:iA + 1],
                                             in1=pa[:, cB, iB:iB + 1])
                        jA = 1 * NH3 * MAXSLOT + hf * MAXSLOT + sA
                        jB = 1 * NH3 * MAXSLOT + hf * MAXSLOT + sB
                        if (cA, sA) == (cB, sB):
                            nc.vector.tensor_copy(out=x2T[:, hf, g:g + 1],
                                                  in_=pa[:, cA, jA:jA + 1])
                        else:
                            nc.vector.tensor_add(out=x2T[:, hf, g:g + 1],
                                                 in0=pa[:, cA, jA:jA + 1],
                                                 in1=pa[:, cB, jB:jB + 1])
                # x3 = x2 * recip_cnt (broadcast graphs along free)
                rc_row = cb.tile([1, G], F32)
                nc.sync.dma_start(out=rc_row[:], in_=t_rcnt[:])
                rcb_ps = psS.tile([P, G], F32, space="PSUM", tag="ps")
                nc.tensor.matmul(out=rcb_ps[:], lhsT=ones_row[:], rhs=rc_row[:],
                                 start=True, stop=True)
                rcb = cb.tile([P, G], F32)
                nc.vector.tensor_copy(out=rcb[:], in_=rcb_ps[:])
                x3T = cb.tile([P, NH3, G], F32)
                for hf in range(NH3):
                    nc.vector.tensor_mul(out=x3T[:, hf, :], in0=x2T[:, hf, :], in1=rcb[:])

                # ---------------- dense tail (transposed: [feat, G]) ----------------
                def load_w(name, r0, rows, c0, cols):
                    wt = sb.tile([rows, cols], F32, tag="tw")
                    nc.sync.dma_start(out=wt[:],
                                      in_=t_tail[name + "_w"][r0:r0 + rows, c0:c0 + cols])
                    return wt

                def load_b(name, c0, rows):
                    bt = sb.tile([rows, 1], F32, tag="tb")
                    nc.sync.dma_start(out=bt[:], in_=t_tail[name + "_b"][c0:c0 + rows, :])
                    return bt

                def dense_T(name, zparts, din, dout, act):
                    """zparts: list of [P or less, G] sbuf tiles (transposed input
                    chunks of size min(P, din)); returns list for dout."""
                    outs = []
                    nko = cdiv(dout, P)
                    nki = cdiv(din, P)
                    for ho in range(nko):
                        wo = min(P, dout - ho * P)
                        pso = psN.tile([P, G], F32, space="PSUM", tag="num")
                        for hi in range(nki):
                            wi = min(P, din - hi * P)
                            wt = load_w(name, hi * P, wi, ho * P, wo)
                            nc.tensor.matmul(out=pso[:wo, :], lhsT=wt[:], rhs=zparts[hi][:wi, :],
                                             start=(hi == 0), stop=(hi == nki - 1))
                        bt = load_b(name, ho * P, wo)
                        ot = sb.tile([P, G], F32, tag=f"t_{name}_{ho}")
                        nc.scalar.activation(ot[:wo, :], pso[:wo, :], act, bias=bt[:, 0:1])
                        outs.append(ot)
                    return outs

                Relu = mybir.ActivationFunctionType.Relu
                Sig = mybir.ActivationFunctionType.Sigmoid
                Copy = mybir.ActivationFunctionType.Copy
                z0 = [x1T[:, 0, :], x1T[:, 1, :], x2T[:, 0, :], x2T[:, 1, :]] if NH3 == 2 \
                    else [x1T[:, 0, :], x2T[:, 0, :]]
                z1 = dense_T("d1", z0, 2 * d3out, 256, Relu)
                z2 = dense_T("d2", z1, 256, 128, Relu)
                z3 = dense_T("d3", z2, 128, 64, Relu)
                gate = dense_T("mean", [x3T[:, h, :] for h in range(NH3)], d3out, 64, Sig)
                z4 = sb.tile([P, G], F32, tag="z4")
                nc.vector.tensor_mul(out=z4[:64, :], in0=z3[0][:64, :], in1=gate[0][:64, :])
                z5 = dense_T("d4", [z4], 64, 64, Relu)
                z6 = dense_T("d5", z5, 64, 128, Relu)
                z7 = dense_T("d6", z6, 128, 256, Relu)
                # final: non-transposed out [G, 128] = z7.T @ d7_w + b
                pso = psN.tile([G, P], F32, space="PSUM", tag="num")
                for hi in range(2):
                    wt = load_w("d7", hi * P, P, 0, P)
                    nc.tensor.matmul(out=pso[:, :], lhsT=z7[hi][:, :], rhs=wt[:],
                                     start=(hi == 0), stop=(hi == 1))
                fb = load_b("d7", 0, P)
                fbb_ps = psS.tile([G, P], F32, space="PSUM", tag="ps")
                onesG = cb.tile([1, G], F32)
                nc.vector.memset(onesG[:], 1.0)
                # bias row [1,128] bcast down G partitions: ones_col(G) x b_row
                fb_row = sb.tile([1, P], F32, tag="fbrow")
                nc.sync.dma_start(out=fb_row[:], in_=t_tail["d7_b"][:, :].rearrange("a b -> b a"))
                nc.tensor.matmul(out=fbb_ps[:, :], lhsT=onesG[:], rhs=fb_row[:],
                                 start=True, stop=True)
                fbb_sb = sb.tile([G, P], F32, tag="fbbsb")
                nc.vector.tensor_copy(out=fbb_sb[:], in_=fbb_ps[:])
                zf = sb.tile([G, P], F32, tag="zf")
                nc.vector.tensor_add(out=zf[:], in0=pso[:], in1=fbb_sb[:])
                nc.sync.dma_start(out=t_out[:, :], in_=zf[:])

    nc.compile()
    return nc


# ======================= kernel entry =======================
G_GRAPHS = 64
LAYER_DIMS = [(128, 64), (64, 128), (128, 256)]
LAST_EXEC_NS = None

_cache = {}


LAST_TRACE_PATH = None


def kernel(x, edge_index, batch, **weights):
    global LAST_EXEC_NS, LAST_TRACE_PATH
    from concourse.bass_utils import run_bass_kernel_spmd
    x = np.asarray(x, np.float32)
    edge_index = np.asarray(edge_index)
    batch = np.asarray(batch)

    pp = preprocess(x, edge_index, batch, G_GRAPHS, LAYER_DIMS)
    in_maps = make_inputs(pp, weights)
    key = (pp["T"], pp["nchunks"], tuple(pp["K_lo"]), tuple(pp["K_hi"]))
    if key not in _cache:
        _cache[key] = build_kernel(pp)
    nc = _cache[key]
    trace = bool(os.environ.get("GAT_TRACE"))
    res = run_bass_kernel_spmd(nc, in_maps, list(range(NCORES)), trace=trace)
    LAST_EXEC_NS = res.exec_time_ns
    if res.instructions_and_trace is not None:
        LAST_TRACE_PATH = res.instructions_and_trace[1]
    return res.results[0]["z_out"].astype(np.float32)

